# revision 1
# baseline (speedup 1.0000x reference)
"""ContextualAttention TRN2 kernel.

Problem (B=4, C=64, H=W=64, K=Q=HW=4096):
    norm_bg = l2norm(bg, axis=C);  norm_fg = l2norm(fg, axis=C)
    att     = softmax_K(norm_bg^T @ norm_fg)        # [B, K, Q]
    out     = fg*(1-mask) + (bg @ att)*mask

Sharding: 8 cores = (batch b in 0..3) x (query half h in 0..1).
Each core sees the full key axis (K=4096) and Q=2048 queries, so the
softmax over K is core-local (no collectives).

Per-core algorithm (everything [C|K on partitions, HW on free]):
  - norms via ones-vector matmuls (partition reduction on PE),
    1/sqrt via exp(-0.5*ln(x)) on ScalarE (Rsqrt activation is banned),
    partition-broadcast of row vectors via ones-row matmuls.
  - scores s[k,q] = bgn^T @ fgn in float32r (1 cyc/row, ~1e-4 rel err),
    32 k-tiles x [128,512] per 512-wide q-tile, grouped 3 PSUM banks at
    a time so one Exp activation covers [128,1536] (ScalarE is the
    bottleneck engine: 8.4M exps/core).
  - softmax denominator for free: bgT is transposed WITH a ones row
    appended, so the re-weighting matmul acc[65,512] = bgT_aug^T @ exp_s
    accumulates both numerator (rows 0..63) and denominator (row 64).
  - epilogue per q-tile: recip on DVE, mask-fold, ones-row broadcast
    matmul, two tensor-tensor ops, DMA out.

This walrus build accepts at most ONE semaphore wait per instruction;
split_multiwaits() post-processes the BIR to hoist extra waits into
single-wait NoOps (see _fix_bir).
"""

import numpy as np

try:
    import concourse.bass as _bass  # noqa: F401
except ImportError:  # pragma: no cover - fallback for odd sys.path setups
    import sys
    for p in ("/opt/trn_rl_repo", "/root/.axon_site/_ro/trn_rl_repo"):
        if p not in sys.path:
            sys.path.insert(0, p)

B, C, H, W = 4, 64, 64, 64
K = H * W              # 4096 keys per batch
QH = K // 2            # 2048 queries per core
NCORES = 8
KT = K // 128          # 32 key tiles
QT = QH // 512         # 4 query tiles per core
GROUPS = [list(range(g * 3, min(KT, g * 3 + 3))) for g in range((KT + 2) // 3)]

_CACHE = {}


def _fix_bir(nc):
    """Hoist extra semaphore waits into single-wait NoOps (this walrus
    supports one wait per instruction) and pin the serialized BIR."""
    import orjson
    bir = orjson.loads(nc.to_json_bytes())
    ctr = 0
    for fn in bir["functions"]:
        for blk in fn["blocks"]:
            out = []
            for inst in blk.get("instructions", []):
                si = inst.get("sync_info")
                ow = (si or {}).get("on_wait") or []
                if len(ow) > 1:
                    for w in ow[:-1]:
                        ctr += 1
                        out.append({
                            "debug": inst.get("debug", 0),
                            "engine": inst["engine"], "ins": [],
                            "name": f"I-wsplit-{ctr}", "opcode": "NoOp",
                            "outs": [],
                            "sync_info": {"on_update": [], "on_wait": [w]},
                        })
                    si["on_wait"] = [ow[-1]]
                out.append(inst)
            blk["instructions"] = out
    fixed = orjson.dumps(bir)
    nc.to_json_bytes = lambda: fixed


def _build_nc():
    import concourse.bass as bass
    import concourse.mybir as mybir
    from concourse import tile

    f32 = mybir.dt.float32
    f32r = mybir.dt.float32r
    bf16 = mybir.dt.bfloat16
    AF = mybir.ActivationFunctionType
    OP = mybir.AluOpType
    mmdt = bf16

    nc = bass.Bass("TRN2", target_bir_lowering=False, debug=False)
    bg_d = nc.dram_tensor("bg", [C, K], f32, kind="ExternalInput")
    fg_d = nc.dram_tensor("fg", [C, QH], f32, kind="ExternalInput")
    mk_d = nc.dram_tensor("mk", [1, QH], f32, kind="ExternalInput")
    id_d = nc.dram_tensor("ident", [128, 128], f32, kind="ExternalInput")
    out_d = nc.dram_tensor("out", [C, QH], f32, kind="ExternalOutput")

    NG = KT // 2  # 16 groups of 2 k-tiles per q-tile

    with tile.TileContext(nc) as tc:
        with (
            tc.tile_pool(name="const", bufs=1) as constp,
            tc.tile_pool(name="sb", bufs=1) as sb,
            tc.tile_pool(name="expp", bufs=4) as expp,
            tc.tile_pool(name="outp", bufs=2) as outp,
            # single PSUM budget for the whole kernel (8 banks):
            #   score [128,1024] x2 = 4, acc [65,512] x2 = 2,
            #   n2 [1,512] = 1, rep/repq [64,512] = 1
            tc.tile_pool(name="mps", bufs=2, space="PSUM") as mps,
            tc.tile_pool(name="accp", bufs=2, space="PSUM") as accp,
            tc.tile_pool(name="n2p", bufs=1, space="PSUM") as n2p,
            tc.tile_pool(name="repp", bufs=1, space="PSUM") as repp,
        ):
            # ---- constants; dummy Ln/Exp prefetch the ACT table set ----
            dumf = constp.tile([1, 8], f32)
            nc.vector.memset(dumf[:], 1.0)
            dumo = constp.tile([1, 8], f32)
            nc.scalar.activation(dumo[:], dumf[:], AF.Ln)
            nc.scalar.activation(dumo[:], dumf[:], AF.Exp)
            ones_col_f = constp.tile([64, 1], f32)
            nc.vector.memset(ones_col_f[:], 1.0)
            ones_col = constp.tile([64, 1], f32r)
            nc.vector.tensor_copy(ones_col[:], ones_col_f[:])
            ones_row_f = constp.tile([1, 64], f32)
            nc.vector.memset(ones_row_f[:], 1.0)
            ones_row = constp.tile([1, 64], f32r)
            nc.vector.tensor_copy(ones_row[:], ones_row_f[:])
            idt = constp.tile([128, 128], f32)

            # ---- input DMAs: fg first (gates q-tile 0) ----
            fgs = sb.tile([64, QH], f32)
            for ch in range(2):
                nc.sync.dma_start(fgs[:, ch * 1024:(ch + 1) * 1024],
                                  fg_d[:, ch * 1024:(ch + 1) * 1024])
            nc.sync.dma_start(idt[:], id_d[:])
            mrow = sb.tile([1, QH], f32)
            nc.sync.dma_start(mrow[:], mk_d[:])
            bgxc = []
            for ch in range(4):
                t = sb.tile([65, 1024], f32, tag=f"bgx{ch}")
                nc.sync.dma_start(t[0:64, :], bg_d[:, ch * 1024:(ch + 1) * 1024])
                nc.vector.memset(t[64:65, :], 1.0)
                bgxc.append(t)

            sqf = sb.tile([64, QH], f32r)
            invn = sb.tile([1, K + QH], f32r)
            bgn = sb.tile([64, K], mmdt)
            fgn = sb.tile([64, QH], mmdt)
            fgm = sb.tile([64, QH], f32)
            bgT = sb.tile([128, KT * 65], mmdt)

            def norm_round(src_ap, dst_off):
                # 512-wide: ones-col matmul, then 1/sqrt = exp(-0.5*ln)
                n2 = n2p.tile([1, 512], f32, tag="n2")
                nc.tensor.matmul(n2[:], ones_col[:], src_ap,
                                 start=True, stop=True)
                lns = outp.tile([1, 512], f32, tag="lns")
                nc.scalar.activation(lns[:], n2[:], AF.Ln)
                nc.scalar.activation(invn[:, dst_off:dst_off + 512],
                                     lns[:], AF.Exp, scale=-0.5)

            def replicate_mul(dst, dst_off, src, src_off, inv_off):
                rep = repp.tile([64, 512], f32, tag="rep")
                nc.tensor.matmul(rep[:], ones_row[:],
                                 invn[0:1, inv_off:inv_off + 512],
                                 start=True, stop=True)
                nc.vector.tensor_mul(dst[:, dst_off:dst_off + 512],
                                     src[0:64, src_off:src_off + 512],
                                     rep[:])

            def bg_chunk_setup(ch):
                bx = bgxc[ch]
                sqb = sb.tile([64, 1024], f32r, tag=f"sqb{ch % 2}")
                nc.vector.tensor_mul(sqb[:], bx[0:64, :], bx[0:64, :])
                for j in range(2):
                    norm_round(sqb[:, j * 512:(j + 1) * 512],
                               ch * 1024 + j * 512)
                for j in range(8):
                    kt = ch * 8 + j
                    trps = mps.tile([128, 65], f32, tag="score")
                    nc.tensor.transpose(trps[:],
                                        bx[:, j * 128:(j + 1) * 128],
                                        idt[0:65, 0:65])
                    nc.vector.tensor_copy(bgT[:, kt * 65:(kt + 1) * 65],
                                          trps[:])
                for j in range(2):
                    replicate_mul(bgn, ch * 1024 + j * 512,
                                  bx, j * 512, ch * 1024 + j * 512)

            def group(qt, g, acc):
                q0 = qt * 512
                kts = [2 * g, 2 * g + 1]
                scp = mps.tile([128, 1024], f32, tag="score")
                for j, kt in enumerate(kts):
                    for h in range(2):
                        nc.tensor.matmul(
                            scp[:, j * 512 + h * 256:j * 512 + (h + 1) * 256],
                            bgn[:, kt * 128:(kt + 1) * 128],
                            fgn[:, q0 + h * 256:q0 + (h + 1) * 256],
                            start=True, stop=True)
                exg = expp.tile([128, 1024], mmdt, tag="exp")
                nc.scalar.activation(exg[:], scp[:], AF.Exp)
                for j, kt in enumerate(kts):
                    nc.tensor.matmul(
                        acc[:], bgT[:, kt * 65:kt * 65 + 65],
                        exg[:, j * 512:(j + 1) * 512],
                        start=(kt == 0), stop=(kt == KT - 1))

            def epilogue(qt, acc):
                q0 = qt * 512
                lnd = outp.tile([1, 512], f32, tag="lnd")
                nc.scalar.activation(lnd[:], acc[64:65, :], AF.Ln)
                rcp = outp.tile([1, 512], f32, tag="rcp")
                nc.scalar.activation(rcp[:], lnd[:], AF.Exp, scale=-1.0)
                mr = outp.tile([1, 512], f32r, tag="mr")
                nc.vector.tensor_mul(mr[:], rcp[:], mrow[0:1, q0:q0 + 512])
                repq = repp.tile([64, 512], f32, tag="rep")
                nc.tensor.matmul(repq[:], ones_row[:], mr[:],
                                 start=True, stop=True)
                rep_sb = outp.tile([64, 512], f32, tag="repsb")
                nc.vector.tensor_copy(rep_sb[:], repq[:])
                ot = outp.tile([64, 512], f32, tag="ot")
                nc.vector.tensor_mul(ot[:], acc[0:64, :], rep_sb[:])
                osb = outp.tile([64, 512], f32, tag="osb")
                nc.vector.tensor_add(osb[:], ot[:], fgm[:, q0:q0 + 512])
                nc.sync.dma_start(out_d[:, q0:q0 + 512], osb[:])

            # ---- fg pipeline (gates everything) ----
            for ch in range(2):
                sl = slice(ch * 1024, (ch + 1) * 1024)
                nc.vector.tensor_mul(sqf[:, sl], fgs[:, sl], fgs[:, sl])
                for j in range(2):
                    norm_round(sqf[:, ch * 1024 + j * 512:
                                    ch * 1024 + (j + 1) * 512],
                               K + ch * 1024 + j * 512)
            replicate_mul(fgn, 0, fgs, 0, K)

            # ---- q-tile 0 interleaved with bg chunk setup ----
            acc0 = accp.tile([65, 512], f32, tag="acc")
            for ch in range(4):
                bg_chunk_setup(ch)
                for g in range(4 * ch, 4 * ch + 4):
                    group(0, g, acc0)
            # remaining fg columns + mask terms (needed from epilogue 0 on)
            for ch in range(1, 4):
                replicate_mul(fgn, ch * 512, fgs, ch * 512, K + ch * 512)
            onem = sb.tile([1, QH], f32)
            nc.vector.tensor_scalar(onem[:], mrow[:], -1.0, 1.0,
                                    OP.mult, OP.add)
            onem_r = sb.tile([1, QH], f32r)
            nc.vector.tensor_copy(onem_r[:], onem[:])
            for ch in range(QT):
                rep = repp.tile([64, 512], f32, tag="rep")
                nc.tensor.matmul(rep[:], ones_row[:],
                                 onem_r[0:1, ch * 512:(ch + 1) * 512],
                                 start=True, stop=True)
                nc.vector.tensor_mul(fgm[:, ch * 512:(ch + 1) * 512],
                                     fgs[:, ch * 512:(ch + 1) * 512], rep[:])
            epilogue(0, acc0)

            # ---- q-tiles 1..3 ----
            for qt in range(1, QT):
                acc = accp.tile([65, 512], f32, tag="acc")
                for g in range(NG):
                    group(qt, g, acc)
                epilogue(qt, acc)

    _fix_bir(nc)
    return nc


def _shard_inputs(background, foreground, mask):
    ident = np.eye(128, dtype=np.float32)
    in_maps = []
    for i in range(NCORES):
        b, h = i // 2, i % 2
        qs = slice(h * QH, (h + 1) * QH)
        in_maps.append({
            "bg": np.ascontiguousarray(
                background[b].reshape(C, K).astype(np.float32)),
            "fg": np.ascontiguousarray(
                foreground[b].reshape(C, K)[:, qs].astype(np.float32)),
            "mk": np.ascontiguousarray(
                mask[b].reshape(1, K)[:, qs].astype(np.float32)),
            "ident": ident,
        })
    return in_maps


def _run(background, foreground, mask, **spmd_kwargs):
    from concourse.bass_utils import run_bass_kernel_spmd
    if "nc" not in _CACHE:
        _CACHE["nc"] = _build_nc()
    nc = _CACHE["nc"]
    in_maps = _shard_inputs(background, foreground, mask)
    res = run_bass_kernel_spmd(nc, in_maps, list(range(NCORES)),
                               **spmd_kwargs)
    out = np.empty((B, C, K), dtype=np.float32)
    for i in range(NCORES):
        b, h = i // 2, i % 2
        out[b, :, h * QH:(h + 1) * QH] = res.results[i]["out"]
    return out.reshape(B, C, H, W), res


def kernel(background, foreground, mask):
    out, _ = _run(background, foreground, mask)
    return out



# revision 5
# speedup vs baseline: 1.2910x; 1.2910x over previous
"""ContextualAttention TRN2 kernel, v2 (mask-sparse + fp8 DoubleRow).

Problem (B=4, C=64, H=W=64, K=Q=HW=4096):
    norm_bg = l2norm(bg, axis=C);  norm_fg = l2norm(fg, axis=C)
    att     = softmax_K(norm_bg^T @ norm_fg)        # [B, K, Q]
    out     = fg*(1-mask) + (bg @ att)*mask

The mask is binary and multiplies `attended`, so attention output is only
needed for masked query columns (~2048 of 4096 per batch).  The host
gathers the masked fg columns (mask-aware query sharding), pads each
core's share to QC=1152, and scatters the attended result back into
out = fg.copy().  8 cores = (batch b) x (query half h); each core sees
all K=4096 keys, so softmax needs no collectives.

Per-core device algorithm (ACT-engine bound, ~8.4M exps halved to 4.7M):
  - inputs are host-prepared layouts: bgt8 [128, 32*80] fp8 = bg^T with a
    ones column (softmax denominator trick) per 128-key tile; bg2/fg2
    bf16 [32, 2, *] channel-split pairs for DoubleRow; fgt8 for fg norms.
  - norms: DVE square+reduce on the transposed tiles gives n2 in column
    layout [128, T], so Ln/Exp cost (T+352)/1.2 ns instead of per-row
    4096-element single-lane passes; 1/sqrt = exp(-0.5*ln(n2) + ln(16))
    (x16 pre-scale keeps fp8 operands out of subnormals; the exp of the
    scores is descaled by 1/256).  Tiny [128,1]->[1,128] DMAs remap the
    inv-norms to DRAM row scratch; a stride-0 (broadcast_to) DRAM-read
    DMA replicates them across partitions for the normalize multiplies
    (GpSimd partition_broadcast fails walrus codegen on this build).
  - scores s[k,q] on PE in fp8 DoubleRow ([32,2,128] x [32,2,384] ->
    [128,384], 0.5 cyc/col); exp on ACT over 4-ktile [128,(4,512),384]
    PSUM groups -> fp8 SBUF; attend acc[65,384] accumulates over k with
    fp8 DoubleRow pairs ([128,2,65] x [128,2,384]).
  - PSUM: group buffer A (4 banks) alternates with B (3 banks), acc 1.
  - epilogue: DVE reciprocal of the denominator row, DRAM-broadcast
    DMA, DVE multiply, DMA out.

This walrus build accepts at most ONE semaphore wait per instruction;
split_multiwaits() post-processes the BIR (see _fix_bir).
"""

import math
import numpy as np
import ml_dtypes

try:
    import concourse.bass as _bass  # noqa: F401
except ImportError:  # pragma: no cover - fallback for odd sys.path setups
    import sys
    for p in ("/opt/trn_rl_repo", "/root/.axon_site/_ro/trn_rl_repo"):
        if p not in sys.path:
            sys.path.insert(0, p)

B, C, H, W = 4, 64, 64, 64
K = H * W              # 4096 keys per batch
KT = K // 128          # 32 key tiles
QC = 1152              # gathered-query capacity per core
QW = 384               # query tile width
QT = QC // QW          # 3 query tiles
NCORES = 8
KTP = 80               # padded bgT tile width (65 used)
SCALE = 16.0           # fp8 operand pre-scale; scores carry SCALE^2
LN_SCALE = math.log(SCALE)
# k-tile groups per q-tile: sizes alternate 4 (PSUM banks 0-3) / 3 (4-6)
GROUP_SIZES = [4, 3, 4, 3, 4, 3, 4, 3, 4]
assert sum(GROUP_SIZES) == KT

FP8 = ml_dtypes.float8_e4m3
BF16 = ml_dtypes.bfloat16

_CACHE = {}


def _fix_bir(nc):
    """Hoist extra semaphore waits into single-wait NoOps (this walrus
    supports one wait per instruction) and pin the serialized BIR."""
    import orjson
    bir = orjson.loads(nc.to_json_bytes())
    ctr = 0
    for fn in bir["functions"]:
        for blk in fn["blocks"]:
            out = []
            for inst in blk.get("instructions", []):
                si = inst.get("sync_info")
                ow = (si or {}).get("on_wait") or []
                if len(ow) > 1:
                    for w in ow[:-1]:
                        ctr += 1
                        out.append({
                            "debug": inst.get("debug", 0),
                            "engine": inst["engine"], "ins": [],
                            "name": f"I-wsplit-{ctr}", "opcode": "NoOp",
                            "outs": [],
                            "sync_info": {"on_update": [], "on_wait": [w]},
                        })
                    si["on_wait"] = [ow[-1]]
                out.append(inst)
            blk["instructions"] = out
    fixed = orjson.dumps(bir)
    nc.to_json_bytes = lambda: fixed


def _build_nc():
    import concourse.bass as bass
    import concourse.mybir as mybir
    from concourse import tile

    f32 = mybir.dt.float32
    bf16 = mybir.dt.bfloat16
    fp8 = mybir.dt.float8e4
    AF = mybir.ActivationFunctionType
    PM = mybir.MatmulPerfMode
    AX = mybir.AxisListType
    OP = mybir.AluOpType

    nc = bass.Bass("TRN2", target_bir_lowering=False, debug=False)
    bgt8_d = nc.dram_tensor("bgt8", [128, KT * KTP], fp8, kind="ExternalInput")
    bg2_d = nc.dram_tensor("bg2", [32, 2 * K], bf16, kind="ExternalInput")
    fg2_d = nc.dram_tensor("fg2", [32, 2 * QC], bf16, kind="ExternalInput")
    fgt8_d = nc.dram_tensor("fgt8", [128, (QC // 128) * KTP], fp8,
                            kind="ExternalInput")
    out_d = nc.dram_tensor("out", [C, QC], f32, kind="ExternalOutput")
    scrB_d = nc.dram_tensor("scrB", [1, K], f32, kind="Internal")
    scrF_d = nc.dram_tensor("scrF", [1, QC], f32, kind="Internal")
    scrR_d = nc.dram_tensor("scrR", [1, QC], f32, kind="Internal")

    FT = QC // 128  # 9 fg norm tiles

    with tile.TileContext(nc) as tc:
        with (
            tc.tile_pool(name="const", bufs=1) as constp,
            tc.tile_pool(name="sb", bufs=1) as sb,
            tc.tile_pool(name="work", bufs=2) as work,
            tc.tile_pool(name="expa", bufs=2) as expap,
            tc.tile_pool(name="expb", bufs=2) as expbp,
            tc.tile_pool(name="outp", bufs=2) as outp,
            # PSUM: A group buffer 4 banks, B group buffer 3 banks, acc 1
            tc.tile_pool(name="mpsA", bufs=1, space="PSUM") as mpsA,
            tc.tile_pool(name="mpsB", bufs=1, space="PSUM") as mpsB,
            tc.tile_pool(name="accp", bufs=1, space="PSUM") as accp,
        ):
            # ---- warmup: ACT table prefetch ----
            dumf = constp.tile([1, 8], f32)
            nc.vector.memset(dumf[:], 1.0)
            dumo = constp.tile([1, 8], f32)
            nc.scalar.activation(dumo[:], dumf[:], AF.Ln)
            nc.scalar.activation(dumo[:], dumf[:], AF.Exp)
            lnsc = constp.tile([128, 1], f32)
            nc.vector.memset(lnsc[:], LN_SCALE)

            # ---- input DMAs (fg first: it gates the first scores) ----
            fgt8 = sb.tile([128, FT, KTP], fp8)
            nc.sync.dma_start(fgt8[:], fgt8_d[:].rearrange(
                "p (t n) -> p t n", t=FT))
            fg2 = sb.tile([32, 2, QC], bf16)
            nc.sync.dma_start(fg2[:], fg2_d[:].rearrange(
                "p (t n) -> p t n", t=2))
            bgt8 = sb.tile([128, KT, KTP], fp8)
            nc.sync.dma_start(bgt8[:], bgt8_d[:].rearrange(
                "p (t n) -> p t n", t=KT))
            bg2 = sb.tile([32, 2, K], bf16)
            for ch in range(4):
                nc.sync.dma_start(
                    bg2[:, :, ch * 1024:(ch + 1) * 1024],
                    bg2_d[:].rearrange("p (t n) -> p t n", t=2)
                    [:, :, ch * 1024:(ch + 1) * 1024])

            bgn8 = sb.tile([32, 2, K], fp8)
            fgn8 = sb.tile([32, 2, QC], fp8)

            # ---- fg normalization chain ----
            sqf = work.tile([128, FT, KTP], f32, tag="sqf")
            nc.vector.tensor_mul(sqf[:], fgt8[:], fgt8[:])
            n2f = work.tile([128, FT], f32, tag="n2f")
            nc.vector.tensor_reduce(n2f[:], sqf[:, :, 0:64], AX.X, OP.add)
            lnf = work.tile([128, FT], f32, tag="lnf")
            nc.scalar.activation(lnf[:], n2f[:], AF.Ln)
            invf = work.tile([128, FT], f32, tag="invf")
            nc.scalar.activation(invf[:], lnf[:], AF.Exp,
                                 scale=-0.5, bias=lnsc[:])
            for t in range(FT):
                nc.sync.dma_start(scrF_d[0:1, 128 * t:128 * (t + 1)],
                                  invf[:, t:t + 1])
            for qt in range(QT):
                q0 = qt * QW
                repf = work.tile([32, QW], f32, tag="repf")
                nc.sync.dma_start(
                    repf[:], scrF_d[0:1, q0:q0 + QW].broadcast_to([32, QW]))
                for i in range(2):
                    nc.vector.tensor_mul(fgn8[:, i, q0:q0 + QW],
                                         fg2[:, i, q0:q0 + QW], repf[:])

            # ---- bg normalization, per chunk of 8 k-tiles ----
            def bg_chunk(ch):
                t0 = ch * 8
                sqb = work.tile([128, 8, KTP], f32, tag="sqb")
                nc.vector.tensor_mul(sqb[:], bgt8[:, t0:t0 + 8, :],
                                     bgt8[:, t0:t0 + 8, :])
                n2b = work.tile([128, 8], f32, tag="n2b")
                nc.vector.tensor_reduce(n2b[:], sqb[:, :, 0:64], AX.X, OP.add)
                lnb = work.tile([128, 8], f32, tag="lnb")
                nc.scalar.activation(lnb[:], n2b[:], AF.Ln)
                invb = work.tile([128, 8], f32, tag="invb")
                nc.scalar.activation(invb[:], lnb[:], AF.Exp,
                                     scale=-0.5, bias=lnsc[:])
                for j in range(8):
                    k0 = 1024 * ch + 128 * j
                    nc.sync.dma_start(scrB_d[0:1, k0:k0 + 128],
                                      invb[:, j:j + 1])
                for r in range(2):
                    k0 = 1024 * ch + 512 * r
                    repb = work.tile([32, 512], f32, tag="repb")
                    nc.sync.dma_start(
                        repb[:],
                        scrB_d[0:1, k0:k0 + 512].broadcast_to([32, 512]))
                    for i in range(2):
                        nc.vector.tensor_mul(bgn8[:, i, k0:k0 + 512],
                                             bg2[:, i, k0:k0 + 512], repb[:])

            # ---- main loop pieces ----
            def group(qt, gi, kt0, acc):
                """Scores + exp + attend for one k-tile group."""
                q0 = qt * QW
                gs = GROUP_SIZES[gi]
                pool, tag, expp = ((mpsA, "A", expap) if gs == 4 else
                                   (mpsB, "B", expbp))
                scp = pool.tile([128, gs, 512], f32, tag=tag)
                for l in range(gs):
                    nc.tensor.matmul(
                        scp[:, l, 0:QW],
                        bgn8[:, :, 128 * (kt0 + l):128 * (kt0 + l + 1)],
                        fgn8[:, :, q0:q0 + QW],
                        start=True, stop=True, perf_mode=PM.DoubleRow)
                ex = expp.tile([128, gs, QW], fp8, tag=f"exp{tag}")
                nc.scalar.activation(ex[:], scp[:, :, 0:QW], AF.Exp,
                                     scale=1.0 / (SCALE * SCALE))
                # attend: DoubleRow pairs (+ one plain matmul for odd gs)
                np_ = gs // 2
                for j in range(np_):
                    nc.tensor.matmul(
                        acc[:], bgt8[:, kt0 + 2 * j:kt0 + 2 * j + 2, 0:65],
                        ex[:, 2 * j:2 * j + 2, :],
                        start=(kt0 + 2 * j == 0),
                        stop=(kt0 + 2 * j + 2 == KT),
                        perf_mode=PM.DoubleRow)
                if gs % 2:
                    nc.tensor.matmul(
                        acc[:], bgt8[:, kt0 + gs - 1, 0:65],
                        ex[:, gs - 1, :],
                        start=(kt0 + gs - 1 == 0),
                        stop=(kt0 + gs == KT))

            def epilogue(qt, acc):
                q0 = qt * QW
                rcp = outp.tile([1, QW], f32, tag="rcp")
                nc.vector.reciprocal(rcp[:], acc[64:65, :])
                nc.sync.dma_start(scrR_d[0:1, q0:q0 + QW], rcp[:])
                rep64 = outp.tile([64, QW], f32, tag="rep64")
                nc.sync.dma_start(
                    rep64[:], scrR_d[0:1, q0:q0 + QW].broadcast_to([64, QW]))
                osb = outp.tile([64, QW], f32, tag="osb")
                nc.vector.tensor_mul(osb[:], acc[0:64, :], rep64[:])
                nc.sync.dma_start(out_d[:, q0:q0 + QW], osb[:])

            # ---- schedule: qtile 0 interleaved with bg chunk setup ----
            kt0s = np.cumsum([0] + GROUP_SIZES[:-1]).tolist()
            bg_chunk(0)
            acc = accp.tile([65, QW], f32, tag="acc")
            for gi, kt0 in enumerate(kt0s):
                # chunk ch covers k-tiles [8ch, 8ch+8); build one ahead
                if gi == 1:
                    bg_chunk(1)
                elif gi == 3:
                    bg_chunk(2)
                elif gi == 5:
                    bg_chunk(3)
                group(0, gi, kt0, acc)
            epilogue(0, acc)

            for qt in range(1, QT):
                acc = accp.tile([65, QW], f32, tag="acc")
                for gi, kt0 in enumerate(kt0s):
                    group(qt, gi, kt0, acc)
                epilogue(qt, acc)

    _fix_bir(nc)
    return nc


def _prep_core_inputs(bg, fgq):
    """Build device layouts for one core.

    bg:  [C, K] float32 (full background for the batch)
    fgq: [C, QC] float32 (gathered+padded foreground queries)
    """
    # bgt8: transposed bg + ones column, padded to KTP, fp8
    bgt = np.zeros((K, KTP), dtype=np.float32)
    bgt[:, 0:C] = bg.T
    bgt[:, C] = 1.0
    bgt8 = np.ascontiguousarray(
        bgt.reshape(KT, 128, KTP).transpose(1, 0, 2).reshape(128, KT * KTP)
    ).astype(FP8)
    # bg2: channel-split pair layout
    bg2 = np.ascontiguousarray(bg.reshape(2, 32, K).transpose(1, 0, 2)
                               .reshape(32, 2 * K)).astype(BF16)
    # fg2
    fg2 = np.ascontiguousarray(fgq.reshape(2, 32, QC).transpose(1, 0, 2)
                               .reshape(32, 2 * QC)).astype(BF16)
    # fgt8: transposed fgq padded (norm source only; col C unused)
    FT = QC // 128
    fgt = np.zeros((QC, KTP), dtype=np.float32)
    fgt[:, 0:C] = fgq.T
    fgt8 = np.ascontiguousarray(
        fgt.reshape(FT, 128, KTP).transpose(1, 0, 2).reshape(128, FT * KTP)
    ).astype(FP8)
    return {"bgt8": bgt8, "bg2": bg2, "fg2": fg2, "fgt8": fgt8}


def _numpy_reference_batch(bg, fg, mask):
    """Full-precision fallback for a single batch (overflow safety)."""
    eps = 1e-12
    nb = bg / np.maximum(np.linalg.norm(bg, axis=0, keepdims=True), eps)
    nf = fg / np.maximum(np.linalg.norm(fg, axis=0, keepdims=True), eps)
    att = nb.T @ nf
    att = np.exp(att - att.max(axis=0, keepdims=True))
    att /= att.sum(axis=0, keepdims=True)
    attended = bg @ att
    return fg * (1.0 - mask) + attended * mask


def _run(background, foreground, mask, **spmd_kwargs):
    from concourse.bass_utils import run_bass_kernel_spmd
    if "nc" not in _CACHE:
        _CACHE["nc"] = _build_nc()
    nc = _CACHE["nc"]

    bg_flat = np.asarray(background, dtype=np.float32).reshape(B, C, K)
    fg_flat = np.asarray(foreground, dtype=np.float32).reshape(B, C, K)
    mk_flat = np.asarray(mask, dtype=np.float32).reshape(B, 1, K)

    idx = []         # per (b, h): gathered query indices
    overflow = []
    for b in range(B):
        ib = np.nonzero(mk_flat[b, 0] != 0.0)[0]
        for h in range(2):
            ih = ib[h::2]
            if len(ih) > QC:
                overflow.append(b)
            idx.append(ih)

    in_maps = []
    for i in range(NCORES):
        b, h = i // 2, i % 2
        ih = idx[2 * b + h][:QC]
        fgq = np.ones((C, QC), dtype=np.float32)
        fgq[:, :len(ih)] = fg_flat[b][:, ih]
        in_maps.append(_prep_core_inputs(bg_flat[b], fgq))

    res = run_bass_kernel_spmd(nc, in_maps, list(range(NCORES)),
                               **spmd_kwargs)

    out = fg_flat.copy()
    for i in range(NCORES):
        b, h = i // 2, i % 2
        ih = idx[2 * b + h][:QC]
        att = np.asarray(res.results[i]["out"], dtype=np.float32)
        out[b][:, ih] = att[:, :len(ih)]
    for b in set(overflow):
        out[b] = _numpy_reference_batch(bg_flat[b], fg_flat[b], mk_flat[b])
    return out.reshape(B, C, H, W), res


def kernel(background, foreground, mask):
    out, _ = _run(background, foreground, mask)
    return out


# revision 6
# speedup vs baseline: 1.3340x; 1.0333x over previous
"""ContextualAttention TRN2 kernel, v3 (mask-sparse + row-tiled scores +
fp8 DoubleRow attend).

Problem (B=4, C=64, H=W=64, K=Q=HW=4096):
    norm_bg = l2norm(bg, axis=C);  norm_fg = l2norm(fg, axis=C)
    att     = softmax_K(norm_bg^T @ norm_fg)        # [B, K, Q]
    out     = fg*(1-mask) + (bg @ att)*mask

The mask is binary and multiplies `attended`, so attention output is only
needed for masked query columns (~2048 of 4096 per batch).  The host
gathers the masked fg columns (mask-aware query sharding), pads each
core's share to QC=1056, and scatters the attended result back into
out = fg.copy().  8 cores = (batch b) x (query half h); each core sees
all K=4096 keys, so softmax needs no collectives.

Per-core device algorithm (ACT-engine bound: 4096*1056 = 4.3M exps):
  - host-prepared layouts: bgt8 [128, 32*80] fp8 = bg^T with a ones
    column (softmax denominator trick); k-tiles interleaved (even tile =
    keys [128m,128m+128), odd = [2048+128m, ...)) so score row-tiling
    and attend DoubleRow pairing agree.  bg_pk/fg_dup bf16 [128, *]:
    key-halves / duplicated channels on the partition halves.
  - norms: DVE square+reduce on the transposed fp8 tiles gives n2 in
    column layout [128, T] so Ln/Exp cost (T+352)/1.2 ns; inv-norms are
    remapped to a DRAM row scratch (one strided DMA per chunk) and read
    back with stride-0 broadcast_to DMAs for the normalize multiplies.
  - scores: PE row-tiling — two concurrent bf16 matmuls at
    tile_position (0,0)/(64,0) (contraction 64 each, disjoint row
    groups) stream one 352-col pass per k-tile PAIR.
  - exp on ACT over [128,(kt,512),352] PSUM groups -> fp8 SBUF.
  - attend acc[65,352] accumulates over k with fp8 DoubleRow pairs
    ([128,2,65] x [128,2,352] -> one 352-col pass per k-tile pair;
    contraction 256 is where DoubleRow genuinely doubles throughput).
  - PSUM: group buffer A (4 banks) alternates with B (2 banks), acc 1.
    Group sizes [2,4,2,4,2,4,2,4,2,4,2] keep k-tile pairs aligned.
  - epilogue: DVE reciprocal of the denominator row, DRAM-broadcast
    DMA, DVE multiply, DMA out.  Small/strided DMAs issue from the idle
    GpSimd queue to keep descriptor generation off the critical path.

This walrus build accepts at most ONE semaphore wait per instruction;
split_multiwaits() post-processes the BIR (see _fix_bir).
"""

import numpy as np
import ml_dtypes

try:
    import concourse.bass as _bass  # noqa: F401
except ImportError:  # pragma: no cover - fallback for odd sys.path setups
    import sys
    for p in ("/opt/trn_rl_repo", "/root/.axon_site/_ro/trn_rl_repo"):
        if p not in sys.path:
            sys.path.insert(0, p)

B, C, H, W = 4, 64, 64, 64
K = H * W              # 4096 keys per batch
KT = K // 128          # 32 key tiles (bgt8 order: interleaved halves)
QC = 1056              # gathered-query capacity per core
QW = 352               # query tile width
QT = QC // QW          # 3 query tiles
FT = 9                 # fg norm tiles (QC padded to 1152 in fgt8)
NCORES = 8
KTP = 80               # padded bgT tile width (65 used)
# k-tile groups per q-tile: sizes alternate 2 (PSUM banks 4-5) / 4 (0-3)
GROUP_SIZES = [2, 4, 2, 4, 2, 4, 2, 4, 2, 4, 2]
assert sum(GROUP_SIZES) == KT

FP8 = ml_dtypes.float8_e4m3
BF16 = ml_dtypes.bfloat16

_CACHE = {}


def _fix_bir(nc):
    """Hoist extra semaphore waits into single-wait NoOps (this walrus
    supports one wait per instruction) and pin the serialized BIR."""
    import orjson
    bir = orjson.loads(nc.to_json_bytes())
    ctr = 0
    for fn in bir["functions"]:
        for blk in fn["blocks"]:
            out = []
            for inst in blk.get("instructions", []):
                si = inst.get("sync_info")
                ow = (si or {}).get("on_wait") or []
                if len(ow) > 1:
                    for w in ow[:-1]:
                        ctr += 1
                        out.append({
                            "debug": inst.get("debug", 0),
                            "engine": inst["engine"], "ins": [],
                            "name": f"I-wsplit-{ctr}", "opcode": "NoOp",
                            "outs": [],
                            "sync_info": {"on_update": [], "on_wait": [w]},
                        })
                    si["on_wait"] = [ow[-1]]
                out.append(inst)
            blk["instructions"] = out
    fixed = orjson.dumps(bir)
    nc.to_json_bytes = lambda: fixed


def _build_nc():
    import concourse.bass as bass
    import concourse.mybir as mybir
    from concourse import tile

    f32 = mybir.dt.float32
    bf16 = mybir.dt.bfloat16
    fp8 = mybir.dt.float8e4
    AF = mybir.ActivationFunctionType
    PM = mybir.MatmulPerfMode
    AX = mybir.AxisListType
    OP = mybir.AluOpType

    nc = bass.Bass("TRN2", target_bir_lowering=False, debug=False)
    bgt8_d = nc.dram_tensor("bgt8", [128, KT * KTP], fp8, kind="ExternalInput")
    bgpk_d = nc.dram_tensor("bgpk", [128, K // 2], bf16, kind="ExternalInput")
    fgdup_d = nc.dram_tensor("fgdup", [128, QC], bf16, kind="ExternalInput")
    fgt8_d = nc.dram_tensor("fgt8", [128, FT * KTP], fp8,
                            kind="ExternalInput")
    out_d = nc.dram_tensor("out", [C, QC], f32, kind="ExternalOutput")
    scrB_d = nc.dram_tensor("scrB", [1, K], f32, kind="Internal")
    scrF_d = nc.dram_tensor("scrF", [1, FT * 128], f32, kind="Internal")
    scrR_d = nc.dram_tensor("scrR", [1, QC], f32, kind="Internal")

    with tile.TileContext(nc) as tc:
        with (
            tc.tile_pool(name="const", bufs=1) as constp,
            tc.tile_pool(name="sb", bufs=1) as sb,
            tc.tile_pool(name="work", bufs=2) as work,
            tc.tile_pool(name="expa", bufs=2) as expap,
            tc.tile_pool(name="expb", bufs=2) as expbp,
            tc.tile_pool(name="outp", bufs=2) as outp,
            # PSUM: A group buffer 4 banks, B group buffer 2, acc 1
            tc.tile_pool(name="mpsA", bufs=1, space="PSUM") as mpsA,
            tc.tile_pool(name="mpsB", bufs=1, space="PSUM") as mpsB,
            tc.tile_pool(name="accp", bufs=1, space="PSUM") as accp,
        ):
            # ---- warmup: ACT table prefetch ----
            dumf = constp.tile([1, 8], f32)
            nc.vector.memset(dumf[:], 1.0)
            dumo = constp.tile([1, 8], f32)
            nc.scalar.activation(dumo[:], dumf[:], AF.Ln)
            nc.scalar.activation(dumo[:], dumf[:], AF.Exp)

            # ---- input DMAs (fg first: it gates the first scores) ----
            fgt8 = sb.tile([128, FT, KTP], fp8)
            nc.sync.dma_start(fgt8[:], fgt8_d[:].rearrange(
                "p (t n) -> p t n", t=FT))
            fgdup = sb.tile([128, QC], bf16)
            nc.sync.dma_start(fgdup[:], fgdup_d[:])
            bgt8 = sb.tile([128, KT, KTP], fp8)
            nc.sync.dma_start(bgt8[:], bgt8_d[:].rearrange(
                "p (t n) -> p t n", t=KT))
            bgpk = sb.tile([128, K // 2], bf16)
            for ch in range(4):
                nc.sync.dma_start(bgpk[:, ch * 512:(ch + 1) * 512],
                                  bgpk_d[:, ch * 512:(ch + 1) * 512])

            bgn2 = sb.tile([128, K // 2], bf16)
            fgn2 = sb.tile([128, QC], bf16)

            # ---- fg normalization chain ----
            sqf = work.tile([128, FT, KTP], f32, tag="sqf")
            nc.vector.tensor_mul(sqf[:], fgt8[:], fgt8[:])
            n2f = work.tile([128, FT], f32, tag="n2f")
            nc.vector.tensor_reduce(n2f[:], sqf[:, :, 0:64], AX.X, OP.add)
            lnf = work.tile([128, FT], f32, tag="lnf")
            nc.scalar.activation(lnf[:], n2f[:], AF.Ln)
            invf = work.tile([128, FT], f32, tag="invf")
            nc.scalar.activation(invf[:], lnf[:], AF.Exp, scale=-0.5)
            nc.gpsimd.dma_start(
                scrF_d[0:1, :].rearrange("o (t p) -> o p t", p=128),
                invf[:])
            repf = work.tile([128, QC], f32, tag="repf")
            nc.gpsimd.dma_start(
                repf[:], scrF_d[0:1, 0:QC].broadcast_to([128, QC]))
            nc.vector.tensor_mul(fgn2[:], fgdup[:], repf[:])

            # ---- bg normalization, per chunk of 8 bgt8 tiles ----
            def bg_chunk(ch):
                t0 = ch * 8
                sqb = work.tile([128, 8, KTP], f32, tag="sqb")
                nc.vector.tensor_mul(sqb[:], bgt8[:, t0:t0 + 8, :],
                                     bgt8[:, t0:t0 + 8, :])
                n2b = work.tile([128, 8], f32, tag="n2b")
                nc.vector.tensor_reduce(n2b[:], sqb[:, :, 0:64], AX.X, OP.add)
                lnb = work.tile([128, 8], f32, tag="lnb")
                nc.scalar.activation(lnb[:], n2b[:], AF.Ln)
                invb = work.tile([128, 8], f32, tag="invb")
                nc.scalar.activation(invb[:], lnb[:], AF.Exp, scale=-0.5)
                # scrB is in bgt8 (interleaved) tile order
                nc.gpsimd.dma_start(
                    scrB_d[0:1, 1024 * ch:1024 * (ch + 1)]
                    .rearrange("o (t p) -> o p t", p=128), invb[:])
                # broadcast back split by half: even tiles -> rows 0:64,
                # odd tiles -> rows 64:128 (keys j / 2048+j of bgpk cols)
                repb = work.tile([128, 512], f32, tag="repb")
                src = scrB_d[0:1, 1024 * ch:1024 * (ch + 1)].rearrange(
                    "o (t two p) -> o two t p", two=2, p=128)
                nc.gpsimd.dma_start(
                    repb[0:64, :].rearrange("p (t n) -> p t n", t=4),
                    src[:, 0].broadcast_to([64, 4, 128]))
                nc.gpsimd.dma_start(
                    repb[64:128, :].rearrange("p (t n) -> p t n", t=4),
                    src[:, 1].broadcast_to([64, 4, 128]))
                nc.vector.tensor_mul(bgn2[:, 512 * ch:512 * (ch + 1)],
                                     bgpk[:, 512 * ch:512 * (ch + 1)],
                                     repb[:])

            # ---- main loop pieces ----
            def group(qt, gi, kt0, acc):
                """Scores + exp + attend for one k-tile group (kt0 even)."""
                q0 = qt * QW
                gs = GROUP_SIZES[gi]
                pool, tag, expp = ((mpsA, "A", expap) if gs == 4 else
                                   (mpsB, "B", expbp))
                scp = pool.tile([128, gs, 512], f32, tag=tag)
                for i in range(gs // 2):
                    m = (kt0 + 2 * i) // 2  # bgpk/bgn2 column tile
                    for half in range(2):
                        nc.tensor.matmul(
                            scp[:, 2 * i + half, 0:QW],
                            bgn2[64 * half:64 * (half + 1),
                                 128 * m:128 * (m + 1)],
                            fgn2[64 * half:64 * (half + 1), q0:q0 + QW],
                            start=True, stop=True,
                            tile_position=(64 * half, 0))
                ex = expp.tile([128, gs, QW], fp8, tag=f"exp{tag}")
                nc.scalar.activation(ex[:], scp[:, :, 0:QW], AF.Exp)
                for i in range(gs // 2):
                    t = kt0 + 2 * i
                    nc.tensor.matmul(
                        acc[:], bgt8[:, t:t + 2, 0:65],
                        ex[:, 2 * i:2 * i + 2, :],
                        start=(t == 0), stop=(t + 2 == KT),
                        perf_mode=PM.DoubleRow)

            def epilogue(qt, acc):
                q0 = qt * QW
                rcp = outp.tile([1, QW], f32, tag="rcp")
                nc.vector.reciprocal(rcp[:], acc[64:65, :])
                nc.gpsimd.dma_start(scrR_d[0:1, q0:q0 + QW], rcp[:])
                rep64 = outp.tile([64, QW], f32, tag="rep64")
                nc.gpsimd.dma_start(
                    rep64[:], scrR_d[0:1, q0:q0 + QW].broadcast_to([64, QW]))
                osb = outp.tile([64, QW], f32, tag="osb")
                nc.vector.tensor_mul(osb[:], acc[0:64, :], rep64[:])
                nc.sync.dma_start(out_d[:, q0:q0 + QW], osb[:])

            # ---- schedule: qtile 0 interleaved with bg chunk setup ----
            kt0s = np.cumsum([0] + GROUP_SIZES[:-1]).tolist()
            bg_chunk(0)
            acc = accp.tile([65, QW], f32, tag="acc")
            for gi, kt0 in enumerate(kt0s):
                # chunk ch covers bgt8 tiles [8ch, 8ch+8); build one ahead
                if gi == 1:
                    bg_chunk(1)
                elif gi == 3:
                    bg_chunk(2)
                elif gi == 5:
                    bg_chunk(3)
                group(0, gi, kt0, acc)
            epilogue(0, acc)

            for qt in range(1, QT):
                acc = accp.tile([65, QW], f32, tag="acc")
                for gi, kt0 in enumerate(kt0s):
                    group(qt, gi, kt0, acc)
                epilogue(qt, acc)

    _fix_bir(nc)
    return nc


def _prep_core_inputs(bg, fgq):
    """Build device layouts for one core.

    bg:  [C, K] float32 (full background for the batch)
    fgq: [C, QC] float32 (gathered+padded foreground queries)
    """
    half = K // 2
    # interleaved key-tile order: tile 2m = keys [128m,128m+128),
    # tile 2m+1 = keys [2048+128m, 2048+128m+128)
    kidx = np.empty((KT, 128), dtype=np.int64)
    for m in range(KT // 2):
        kidx[2 * m] = np.arange(128 * m, 128 * (m + 1))
        kidx[2 * m + 1] = half + np.arange(128 * m, 128 * (m + 1))
    # bgt8: transposed bg + ones column, padded to KTP, fp8
    bgt = np.zeros((KT, 128, KTP), dtype=np.float32)
    bgt[:, :, 0:C] = bg.T[kidx.reshape(-1)].reshape(KT, 128, C)
    bgt[:, :, C] = 1.0
    bgt8 = np.ascontiguousarray(
        bgt.transpose(1, 0, 2).reshape(128, KT * KTP)).astype(FP8)
    # bgpk: key-halves stacked on partition halves
    bgpk = np.concatenate([bg[:, :half], bg[:, half:]], axis=0).astype(BF16)
    # fgdup: duplicated channels
    fgdup = np.concatenate([fgq, fgq], axis=0).astype(BF16)
    # fgt8: transposed fgq (padded to FT*128 queries) for norms
    fgt = np.zeros((FT * 128, KTP), dtype=np.float32)
    fgt[0:QC, 0:C] = fgq.T
    fgt[QC:, 0:C] = 1.0
    fgt8 = np.ascontiguousarray(
        fgt.reshape(FT, 128, KTP).transpose(1, 0, 2).reshape(128, FT * KTP)
    ).astype(FP8)
    return {"bgt8": bgt8, "bgpk": bgpk, "fgdup": fgdup, "fgt8": fgt8}


def _numpy_reference_batch(bg, fg, mask):
    """Full-precision fallback for a single batch (overflow safety)."""
    eps = 1e-12
    nb = bg / np.maximum(np.linalg.norm(bg, axis=0, keepdims=True), eps)
    nf = fg / np.maximum(np.linalg.norm(fg, axis=0, keepdims=True), eps)
    att = nb.T @ nf
    att = np.exp(att - att.max(axis=0, keepdims=True))
    att /= att.sum(axis=0, keepdims=True)
    attended = bg @ att
    return fg * (1.0 - mask) + attended * mask


def _run(background, foreground, mask, **spmd_kwargs):
    from concourse.bass_utils import run_bass_kernel_spmd
    if "nc" not in _CACHE:
        _CACHE["nc"] = _build_nc()
    nc = _CACHE["nc"]

    bg_flat = np.asarray(background, dtype=np.float32).reshape(B, C, K)
    fg_flat = np.asarray(foreground, dtype=np.float32).reshape(B, C, K)
    mk_flat = np.asarray(mask, dtype=np.float32).reshape(B, 1, K)

    idx = []         # per (b, h): gathered query indices
    overflow = []
    for b in range(B):
        ib = np.nonzero(mk_flat[b, 0] != 0.0)[0]
        for h in range(2):
            ih = ib[h::2]
            if len(ih) > QC:
                overflow.append(b)
            idx.append(ih)

    in_maps = []
    for i in range(NCORES):
        b, h = i // 2, i % 2
        ih = idx[2 * b + h][:QC]
        fgq = np.ones((C, QC), dtype=np.float32)
        fgq[:, :len(ih)] = fg_flat[b][:, ih]
        in_maps.append(_prep_core_inputs(bg_flat[b], fgq))

    res = run_bass_kernel_spmd(nc, in_maps, list(range(NCORES)),
                               **spmd_kwargs)

    out = fg_flat.copy()
    for i in range(NCORES):
        b, h = i // 2, i % 2
        ih = idx[2 * b + h][:QC]
        att = np.asarray(res.results[i]["out"], dtype=np.float32)
        out[b][:, ih] = att[:, :len(ih)]
    for b in set(overflow):
        out[b] = _numpy_reference_batch(bg_flat[b], fg_flat[b], mk_flat[b])
    return out.reshape(B, C, H, W), res


def kernel(background, foreground, mask):
    out, _ = _run(background, foreground, mask)
    return out


# revision 14
# speedup vs baseline: 1.3702x; 1.0272x over previous
"""ContextualAttention TRN2 kernel, v3 (mask-sparse + row-tiled scores +
fp8 DoubleRow attend).

Problem (B=4, C=64, H=W=64, K=Q=HW=4096):
    norm_bg = l2norm(bg, axis=C);  norm_fg = l2norm(fg, axis=C)
    att     = softmax_K(norm_bg^T @ norm_fg)        # [B, K, Q]
    out     = fg*(1-mask) + (bg @ att)*mask

The mask is binary and multiplies `attended`, so attention output is only
needed for masked query columns (~2048 of 4096 per batch).  The host
gathers the masked fg columns (mask-aware query sharding), pads each
core's share to QC=1056, and scatters the attended result back into
out = fg.copy().  8 cores = (batch b) x (query half h); each core sees
all K=4096 keys, so softmax needs no collectives.

Per-core device algorithm (ACT-engine bound: 4096*1056 = 4.3M exps):
  - host-prepared layouts: bgt8 [128, 32*80] fp8 = bg^T with a ones
    column (softmax denominator trick); k-tiles interleaved (even tile =
    keys [128m,128m+128), odd = [2048+128m, ...)) so score row-tiling
    and attend DoubleRow pairing agree.  bg_pk/fg_dup bf16 [128, *]:
    key-halves / duplicated channels on the partition halves.
  - norms: DVE square+reduce on the transposed fp8 tiles gives n2 in
    column layout [128, T] so Ln/Exp cost (T+352)/1.2 ns; inv-norms are
    remapped to a DRAM row scratch (one strided DMA per chunk), read
    back contiguously as f32r rows, and replicated across partitions by
    ones-row PE matmuls into the spare PSUM bank (col-tiled (0,0)/(0,64)
    pair for the bg key-halves).  Stride-0 broadcast DMAs proved slow
    (descriptor-heavy, long latency chains on the DMA queue).
  - scores: PE row-tiling — two concurrent bf16 matmuls at
    tile_position (0,0)/(64,0) (contraction 64 each, disjoint row
    groups) stream one 352-col pass per k-tile PAIR.
  - exp on ACT over [128,(kt,512),352] PSUM groups -> fp8 SBUF.
  - attend acc[65,352] accumulates over k with fp8 DoubleRow pairs
    ([128,2,65] x [128,2,352] -> one 352-col pass per k-tile pair;
    contraction 256 is where DoubleRow genuinely doubles throughput).
  - PSUM: group buffer A (4 banks) alternates with B (2 banks), acc 1.
    Group sizes [2,4,2,4,2,4,2,4,2,4,2] keep k-tile pairs aligned.
  - epilogue: DVE reciprocal of the denominator row (f32r), ones-row
    replicate matmul, DVE multiply, DMA out.  Remap/readback DMAs issue
    from the idle GpSimd queue to keep descriptor generation off the
    critical path.

This walrus build accepts at most ONE semaphore wait per instruction;
split_multiwaits() post-processes the BIR (see _fix_bir).
"""

import numpy as np
import ml_dtypes

try:
    import concourse.bass as _bass  # noqa: F401
except ImportError:  # pragma: no cover - fallback for odd sys.path setups
    import sys
    for p in ("/opt/trn_rl_repo", "/root/.axon_site/_ro/trn_rl_repo"):
        if p not in sys.path:
            sys.path.insert(0, p)

B, C, H, W = 4, 64, 64, 64
K = H * W              # 4096 keys per batch
KT = K // 128          # 32 key tiles (bgt8 order: interleaved halves)
QC = 1056              # gathered-query capacity per core
QW = 352               # query tile width
QT = QC // QW          # 3 query tiles
FT = 9                 # fg norm tiles (QC padded to 1152 in fgt8)
NCORES = 8
KTP = 80               # padded bgT tile width (65 used)
# k-tile groups per q-tile: sizes alternate 2 (PSUM banks 4-5) / 4 (0-3)
GROUP_SIZES = [2, 4, 2, 4, 2, 4, 2, 4, 2, 4, 2]
assert sum(GROUP_SIZES) == KT

FP8 = ml_dtypes.float8_e4m3
BF16 = ml_dtypes.bfloat16

_CACHE = {}


def _fix_bir(nc):
    """Hoist extra semaphore waits into single-wait NoOps (this walrus
    supports one wait per instruction) and pin the serialized BIR."""
    import orjson
    bir = orjson.loads(nc.to_json_bytes())
    ctr = 0
    for fn in bir["functions"]:
        for blk in fn["blocks"]:
            out = []
            for inst in blk.get("instructions", []):
                si = inst.get("sync_info")
                ow = (si or {}).get("on_wait") or []
                if len(ow) > 1:
                    for w in ow[:-1]:
                        ctr += 1
                        out.append({
                            "debug": inst.get("debug", 0),
                            "engine": inst["engine"], "ins": [],
                            "name": f"I-wsplit-{ctr}", "opcode": "NoOp",
                            "outs": [],
                            "sync_info": {"on_update": [], "on_wait": [w]},
                        })
                    si["on_wait"] = [ow[-1]]
                out.append(inst)
            blk["instructions"] = out
    fixed = orjson.dumps(bir)
    nc.to_json_bytes = lambda: fixed


def _build_nc():
    import concourse.bass as bass
    import concourse.mybir as mybir
    from concourse import tile

    f32 = mybir.dt.float32
    f32r = mybir.dt.float32r
    bf16 = mybir.dt.bfloat16
    fp8 = mybir.dt.float8e4
    AF = mybir.ActivationFunctionType
    PM = mybir.MatmulPerfMode
    AX = mybir.AxisListType
    OP = mybir.AluOpType

    nc = bass.Bass("TRN2", target_bir_lowering=False, debug=False)
    bgt8_d = nc.dram_tensor("bgt8", [128, KT * KTP], fp8, kind="ExternalInput")
    bgpk_d = nc.dram_tensor("bgpk", [128, K // 2], bf16, kind="ExternalInput")
    fgdup_d = nc.dram_tensor("fgdup", [128, QC], bf16, kind="ExternalInput")
    fgt8_d = nc.dram_tensor("fgt8", [128, FT * KTP], fp8,
                            kind="ExternalInput")
    sel_d = nc.dram_tensor("sel", [2, 128], f32r, kind="ExternalInput")
    out_d = nc.dram_tensor("out", [C, QC], f32, kind="ExternalOutput")
    scrB_d = nc.dram_tensor("scrB", [1, K], f32, kind="Internal")
    scrF_d = nc.dram_tensor("scrF", [1, FT * 128], f32, kind="Internal")


    with tile.TileContext(nc) as tc:
        with (
            tc.tile_pool(name="const", bufs=1) as constp,
            tc.tile_pool(name="sb", bufs=1) as sb,
            tc.tile_pool(name="work", bufs=2) as work,
            tc.tile_pool(name="expa", bufs=2) as expap,
            tc.tile_pool(name="expb", bufs=2) as expbp,
            tc.tile_pool(name="outp", bufs=2) as outp,
            # PSUM: A group buffer 4 banks, B group buffer 2, acc 1
            tc.tile_pool(name="mpsA", bufs=1, space="PSUM") as mpsA,
            tc.tile_pool(name="mpsB", bufs=1, space="PSUM") as mpsB,
            tc.tile_pool(name="accp", bufs=1, space="PSUM") as accp,
            tc.tile_pool(name="repp", bufs=1, space="PSUM") as repp,
        ):
            # ---- warmup: ACT table prefetch ----
            dumf = constp.tile([1, 8], f32)
            nc.vector.memset(dumf[:], 1.0)
            dumo = constp.tile([1, 8], f32)
            nc.scalar.activation(dumo[:], dumf[:], AF.Ln)
            nc.scalar.activation(dumo[:], dumf[:], AF.Exp)
            ones_f = constp.tile([1, 128], f32)
            nc.vector.memset(ones_f[:], 1.0)
            ones_r = constp.tile([1, 128], f32r)
            nc.vector.tensor_copy(ones_r[:], ones_f[:])
            # selector: row0 -> partitions 0:64, row1 -> partitions 64:128
            sel_r = constp.tile([2, 128], f32r)
            nc.sync.dma_start(sel_r[:], sel_d[:])

            # ---- input DMAs (fg first: it gates the first scores) ----
            fgt8 = sb.tile([128, FT, KTP], fp8)
            nc.sync.dma_start(fgt8[:], fgt8_d[:].rearrange(
                "p (t n) -> p t n", t=FT))
            fgdup = sb.tile([128, QC], bf16)
            nc.sync.dma_start(fgdup[:], fgdup_d[:])
            bgt8 = sb.tile([128, KT, KTP], fp8)
            nc.sync.dma_start(bgt8[:], bgt8_d[:].rearrange(
                "p (t n) -> p t n", t=KT))
            bgpk = sb.tile([128, K // 2], bf16)
            nc.sync.dma_start(bgpk[:], bgpk_d[:])

            bgn2 = sb.tile([128, K // 2], bf16)
            fgn2 = sb.tile([128, QC], bf16)

            # ---- fg normalization chain ----
            sqf = work.tile([128, FT, KTP], f32, tag="sqf")
            nc.vector.tensor_mul(sqf[:], fgt8[:], fgt8[:])
            n2f = work.tile([128, FT], f32, tag="n2f")
            nc.vector.tensor_reduce(n2f[:], sqf[:, :, 0:64], AX.X, OP.add)
            lnf = work.tile([128, FT], f32, tag="lnf")
            nc.scalar.activation(lnf[:], n2f[:], AF.Ln)
            invf = work.tile([128, FT], f32, tag="invf")
            nc.scalar.activation(invf[:], lnf[:], AF.Exp, scale=-0.5)
            nc.gpsimd.dma_start(
                scrF_d[0:1, :].rearrange("o (t p) -> o p t", p=128),
                invf[:])
            invfrow = work.tile([1, QC], f32r, tag="invfrow")
            nc.gpsimd.dma_start(invfrow[:], scrF_d[0:1, 0:QC])
            for qt in range(QT):
                q0 = qt * QW
                repf = repp.tile([128, 512], f32, tag="rep")
                nc.tensor.matmul(repf[:, 0:QW], ones_r[:, 0:128],
                                 invfrow[:, q0:q0 + QW],
                                 start=True, stop=True)
                nc.vector.tensor_mul(fgn2[:, q0:q0 + QW],
                                     fgdup[:, q0:q0 + QW], repf[:, 0:QW])

            # ---- bg normalization, per chunk of 8 bgt8 tiles ----
            def bg_chunk(ch):
                t0 = ch * 8
                sqb = work.tile([128, 8, KTP], f32, tag="sqb")
                nc.vector.tensor_mul(sqb[:], bgt8[:, t0:t0 + 8, :],
                                     bgt8[:, t0:t0 + 8, :])
                n2b = work.tile([128, 8], f32, tag="n2b")
                nc.vector.tensor_reduce(n2b[:], sqb[:, :, 0:64], AX.X, OP.add)
                lnb = work.tile([128, 8], f32, tag="lnb")
                nc.scalar.activation(lnb[:], n2b[:], AF.Ln)
                invb = work.tile([128, 8], f32, tag="invb")
                nc.scalar.activation(invb[:], lnb[:], AF.Exp, scale=-0.5)
                # scrB per chunk: [even-tile keys 512 | odd-tile keys 512]
                for half in range(2):
                    nc.gpsimd.dma_start(
                        scrB_d[0:1, 1024 * ch + 512 * half:
                               1024 * ch + 512 * (half + 1)]
                        .rearrange("o (t p) -> o p t", p=128),
                        invb[:, half::2])
                # read back as [2, 512]: row0 = even tiles, row1 = odd
                invbrow = work.tile([2, 512], f32r, tag="invbrow")
                nc.gpsimd.dma_start(
                    invbrow[:],
                    scrB_d[0:1, 1024 * ch:1024 * (ch + 1)]
                    .rearrange("o (two n) -> (o two) n", two=2))
                # replicate via selector matmul: partitions 0:64 get row0
                # (first-half keys), 64:128 get row1 (second-half keys)
                repb = repp.tile([128, 512], f32, tag="rep")
                nc.tensor.matmul(repb[:], sel_r[:], invbrow[:],
                                 start=True, stop=True)
                nc.vector.tensor_mul(bgn2[:, 512 * ch:512 * (ch + 1)],
                                     bgpk[:, 512 * ch:512 * (ch + 1)],
                                     repb[:])

            # ---- main loop pieces ----
            def group(qt, gi, kt0, acc):
                """Scores + exp + attend for one k-tile group (kt0 even)."""
                q0 = qt * QW
                gs = GROUP_SIZES[gi]
                pool, tag, expp = ((mpsA, "A", expap) if gs == 4 else
                                   (mpsB, "B", expbp))
                scp = pool.tile([128, gs, 512], f32, tag=tag)
                for i in range(gs // 2):
                    m = (kt0 + 2 * i) // 2  # bgpk/bgn2 column tile
                    for half in range(2):
                        nc.tensor.matmul(
                            scp[:, 2 * i + half, 0:QW],
                            bgn2[64 * half:64 * (half + 1),
                                 128 * m:128 * (m + 1)],
                            fgn2[64 * half:64 * (half + 1), q0:q0 + QW],
                            start=True, stop=True,
                            tile_position=(64 * half, 0))
                ex = expp.tile([128, gs, QW], fp8, tag=f"exp{tag}")
                nc.scalar.activation(ex[:], scp[:, :, 0:QW], AF.Exp)
                for i in range(gs // 2):
                    t = kt0 + 2 * i
                    nc.tensor.matmul(
                        acc[:], bgt8[:, t:t + 2, 0:65],
                        ex[:, 2 * i:2 * i + 2, :],
                        start=(t == 0), stop=(t + 2 == KT),
                        perf_mode=PM.DoubleRow)

            def epilogue(qt, acc):
                q0 = qt * QW
                rcp = outp.tile([1, QW], f32r, tag="rcp")
                with nc.allow_low_precision(reason="f32r view of fp32 bits"):
                    nc.vector.reciprocal(rcp[:], acc[64:65, :])
                rep64 = repp.tile([128, 512], f32, tag="rep")
                nc.tensor.matmul(rep64[0:64, 0:QW], ones_r[:, 0:64], rcp[:],
                                 start=True, stop=True)
                rep64_sb = outp.tile([64, QW], f32, tag="rep64sb")
                nc.vector.tensor_copy(rep64_sb[:], rep64[0:64, 0:QW])
                osb = outp.tile([64, QW], f32, tag="osb")
                nc.vector.tensor_mul(osb[:], acc[0:64, :], rep64_sb[:])
                nc.sync.dma_start(out_d[:, q0:q0 + QW], osb[:])

            # ---- schedule: qtile 0 interleaved with bg chunk setup ----
            kt0s = np.cumsum([0] + GROUP_SIZES[:-1]).tolist()
            bg_chunk(0)
            acc = accp.tile([65, QW], f32, tag="acc")
            for gi, kt0 in enumerate(kt0s):
                # chunk ch covers bgt8 tiles [8ch, 8ch+8); build one ahead
                if gi == 1:
                    bg_chunk(1)
                elif gi == 3:
                    bg_chunk(2)
                elif gi == 5:
                    bg_chunk(3)
                group(0, gi, kt0, acc)
            epilogue(0, acc)

            for qt in range(1, QT):
                acc = accp.tile([65, QW], f32, tag="acc")
                for gi, kt0 in enumerate(kt0s):
                    group(qt, gi, kt0, acc)
                epilogue(qt, acc)

    _fix_bir(nc)
    return nc


def _prep_core_inputs(bg, fgq):
    """Build device layouts for one core.

    bg:  [C, K] float32 (full background for the batch)
    fgq: [C, QC] float32 (gathered+padded foreground queries)
    """
    half = K // 2
    # interleaved key-tile order: tile 2m = keys [128m,128m+128),
    # tile 2m+1 = keys [2048+128m, 2048+128m+128)
    kidx = np.empty((KT, 128), dtype=np.int64)
    for m in range(KT // 2):
        kidx[2 * m] = np.arange(128 * m, 128 * (m + 1))
        kidx[2 * m + 1] = half + np.arange(128 * m, 128 * (m + 1))
    # bgt8: transposed bg + ones column, padded to KTP, fp8
    bgt = np.zeros((KT, 128, KTP), dtype=np.float32)
    bgt[:, :, 0:C] = bg.T[kidx.reshape(-1)].reshape(KT, 128, C)
    bgt[:, :, C] = 1.0
    bgt8 = np.ascontiguousarray(
        bgt.transpose(1, 0, 2).reshape(128, KT * KTP)).astype(FP8)
    # bgpk: key-halves stacked on partition halves
    bgpk = np.concatenate([bg[:, :half], bg[:, half:]], axis=0).astype(BF16)
    # fgdup: duplicated channels
    fgdup = np.concatenate([fgq, fgq], axis=0).astype(BF16)
    # fgt8: transposed fgq (padded to FT*128 queries) for norms
    fgt = np.zeros((FT * 128, KTP), dtype=np.float32)
    fgt[0:QC, 0:C] = fgq.T
    fgt[QC:, 0:C] = 1.0
    fgt8 = np.ascontiguousarray(
        fgt.reshape(FT, 128, KTP).transpose(1, 0, 2).reshape(128, FT * KTP)
    ).astype(FP8)
    sel = np.zeros((2, 128), dtype=np.float32)
    sel[0, 0:64] = 1.0
    sel[1, 64:128] = 1.0
    return {"bgt8": bgt8, "bgpk": bgpk, "fgdup": fgdup, "fgt8": fgt8,
            "sel": sel}


def _numpy_reference_batch(bg, fg, mask):
    """Full-precision fallback for a single batch (overflow safety)."""
    eps = 1e-12
    nb = bg / np.maximum(np.linalg.norm(bg, axis=0, keepdims=True), eps)
    nf = fg / np.maximum(np.linalg.norm(fg, axis=0, keepdims=True), eps)
    att = nb.T @ nf
    att = np.exp(att - att.max(axis=0, keepdims=True))
    att /= att.sum(axis=0, keepdims=True)
    attended = bg @ att
    return fg * (1.0 - mask) + attended * mask


def _run(background, foreground, mask, **spmd_kwargs):
    from concourse.bass_utils import run_bass_kernel_spmd
    if "nc" not in _CACHE:
        _CACHE["nc"] = _build_nc()
    nc = _CACHE["nc"]

    bg_flat = np.asarray(background, dtype=np.float32).reshape(B, C, K)
    fg_flat = np.asarray(foreground, dtype=np.float32).reshape(B, C, K)
    mk_flat = np.asarray(mask, dtype=np.float32).reshape(B, 1, K)

    idx = []         # per (b, h): gathered query indices
    overflow = []
    for b in range(B):
        ib = np.nonzero(mk_flat[b, 0] != 0.0)[0]
        for h in range(2):
            ih = ib[h::2]
            if len(ih) > QC:
                overflow.append(b)
            idx.append(ih)

    in_maps = []
    for i in range(NCORES):
        b, h = i // 2, i % 2
        ih = idx[2 * b + h][:QC]
        fgq = np.ones((C, QC), dtype=np.float32)
        fgq[:, :len(ih)] = fg_flat[b][:, ih]
        in_maps.append(_prep_core_inputs(bg_flat[b], fgq))

    res = run_bass_kernel_spmd(nc, in_maps, list(range(NCORES)),
                               **spmd_kwargs)

    out = fg_flat.copy()
    for i in range(NCORES):
        b, h = i // 2, i % 2
        ih = idx[2 * b + h][:QC]
        att = np.asarray(res.results[i]["out"], dtype=np.float32)
        out[b][:, ih] = att[:, :len(ih)]
    for b in set(overflow):
        out[b] = _numpy_reference_batch(bg_flat[b], fg_flat[b], mk_flat[b])
    return out.reshape(B, C, H, W), res


def kernel(background, foreground, mask):
    out, _ = _run(background, foreground, mask)
    return out
